# revision 12
# baseline (speedup 1.0000x reference)
# Distributed Trainium2 kernel for the GQA attention block
# (nn_Attention_52621939311076).
#
# Sharding: tensor-parallel over heads across 8 NeuronCores. Core c owns
# q-heads [8c, 8c+8) and kv-head c (GQA group stays local). x is replicated,
# wq/wk/wv are sharded on the output dim, wo on the input dim; partial wo
# outputs are summed with an on-device ReduceScatter and the rank slices are
# concatenated on the host.
#
# Everything on device lives in a transposed [feature, seq] layout so that no
# on-chip transposes are needed anywhere:
#   - projections produce Q^T/K^T (head_dim on partitions) and V in [s, d],
#   - RMSNorm reduction over head_dim uses a ones-matmul (partition reduce),
#   - RoPE pairs are (even, odd) partition halves via a host-side permutation
#     of the wq/wk output dims,
#   - attention computes S^T = K^T.T-stationary @ Q^T, softmax row sums via a
#     ones-matmul, O^T = V-stationary @ P^T,
#   - the wo matmul consumes O^T directly.
# Matmuls run in bf16 (4x the fp32 TensorE rate), accumulating in fp32 PSUM.
import numpy as np
import ml_dtypes

import concourse.bass as bass
import concourse.bacc as bacc
import concourse.mybir as mybir
import concourse.tile as tile
from concourse.bass_utils import run_bass_kernel_spmd

BF16 = mybir.dt.bfloat16
F32 = mybir.dt.float32
NPBF16 = ml_dtypes.bfloat16

N_CORES = 8
S = 2048          # sequence length
D = 5120          # model dim
H = 64            # q heads (global)
KVH = 8           # kv heads (global)
HD = 128          # head dim
HQ = H // N_CORES  # q heads per core
DC = D // 128     # contraction chunks for the projections
SB = S // 128     # 128-row seq blocks
NG = S // 512     # 512-col seq groups
DG = D // 512     # 512-col output groups for wo
EPS = 1e-6

_cache = {}


def _build(causal: bool):
    nc = bacc.Bacc("TRN2", target_bir_lowering=False, debug=False,
                   num_devices=N_CORES)

    xt_e = nc.dram_tensor("xt", [DC, NG, 128, 512], BF16, kind="ExternalInput")
    wq_e = nc.dram_tensor("wq", [DC, HQ, 128, 128], BF16, kind="ExternalInput")
    wk_e = nc.dram_tensor("wk", [DC, 128, 128], BF16, kind="ExternalInput")
    wv_e = nc.dram_tensor("wv", [DC, 128, 128], BF16, kind="ExternalInput")
    wo_e = nc.dram_tensor("wo", [HQ, DG, 128, 512], BF16, kind="ExternalInput")
    cos_e = nc.dram_tensor("cos", [128, S], BF16, kind="ExternalInput")
    sin_e = nc.dram_tensor("sin", [128, S], BF16, kind="ExternalInput")
    swp_e = nc.dram_tensor("swp", [128, 128], BF16, kind="ExternalInput")
    qw_e = nc.dram_tensor("qw", [128, 1], F32, kind="ExternalInput")
    kw_e = nc.dram_tensor("kw", [128, 1], F32, kind="ExternalInput")
    if causal:
        mask_e = nc.dram_tensor("mask", [SB, 128, 128], F32, kind="ExternalInput")
    else:
        mask_e = nc.dram_tensor("mask", [SB, NG, 128, 512], F32,
                                kind="ExternalInput")
    out_e = nc.dram_tensor("out", [S // N_CORES, D], F32, kind="ExternalOutput")

    mult = mybir.AluOpType.mult
    Exp = mybir.ActivationFunctionType.Exp
    Sqrt = mybir.ActivationFunctionType.Sqrt
    Square = mybir.ActivationFunctionType.Square

    with tile.TileContext(nc) as tc, \
         tc.tile_pool(name="persist", bufs=1) as persist:
        def single(shape, dtype, name):
            return persist.tile(shape, dtype, name=name, tag=name)

        # ---- persistent SBUF tensors -------------------------------------
        QR = single([128, HQ * S], BF16, "QR")     # roped q, [d, s] per head
        KR = single([128, S], BF16, "KR")          # roped k, [d, s]
        Vsd = single([128, S], BF16, "Vsd")        # v in [s, d], s-block b at cols b*128
        OT = single([128, HQ * S], BF16, "OT")     # attn out^T, [d, s] per head
        cosT = single([128, S], BF16, "cosT")   # cos duplicated on both halves
        sinT = single([128, S], BF16, "sinT")   # [-sin; +sin]
        swp_t = single([128, 128], BF16, "swp_t")
        qw_t = single([128, 1], F32, "qw_t")
        kw_t = single([128, 1], F32, "kw_t")
        ones_c = single([128, 1], BF16, "ones_c")  # column of ones
        ones_r = single([1, 128], BF16, "ones_r")  # row of ones
        eps_t = single([128, 1], F32, "eps_t")
        if causal:
            maskT = single([128, SB * 128], F32, "maskT")

        nc.sync.dma_start(out=cosT[:, :], in_=cos_e[:, :])
        nc.sync.dma_start(out=sinT[:, :], in_=sin_e[:, :])
        nc.sync.dma_start(out=swp_t[:, :], in_=swp_e[:, :])
        nc.sync.dma_start(out=qw_t[:, :], in_=qw_e[:, :])
        nc.sync.dma_start(out=kw_t[:, :], in_=kw_e[:, :])
        nc.vector.memset(ones_c[:, :], 1.0)
        nc.vector.memset(ones_r[:, :], 1.0)
        nc.vector.memset(eps_t[:, :], EPS)
        if causal:
            for b in range(SB):
                nc.sync.dma_start(out=maskT[:, b * 128:(b + 1) * 128],
                                  in_=mask_e[b])

        # ---- stage 1+2: projections + rmsnorm + rope ---------------------
        def norm_rope(pj, w_ap, dst, dst_cols, sg, sqp, ssp, bcp, swpp, stats,
                      tmps):
            """pj: PSUM [128,512] projection block; writes roped dst[:, dst_cols]."""
            sq = sqp.tile([128, 512], BF16, tag="sq")
            nc.scalar.activation(sq[:, :], pj[:, :], Square)
            ss = ssp.tile([1, 512], F32, tag="ss")
            nc.tensor.matmul(ss[:, :], ones_c[:, :], sq[:, :], start=True, stop=True)
            ssb = stats.tile([1, 512], BF16, tag="ssb")
            nc.vector.tensor_copy(ssb[:, :], ss[:, :])
            bc = bcp.tile([128, 512], F32, tag="bc")
            nc.tensor.matmul(bc[:, :], ones_r[:, :], ssb[:, :], start=True, stop=True)
            rstd = stats.tile([128, 512], F32, tag="rstd")
            nc.scalar.activation(rstd[:, :], bc[:, :], Sqrt, bias=eps_t[:, :],
                                 scale=1.0 / HD)
            rec = stats.tile([128, 512], F32, tag="rec")
            nc.vector.reciprocal(rec[:, :], rstd[:, :])
            qn = stats.tile([128, 512], BF16, tag="qn")
            # qn = (pj * w) * rec  -- normalized, weighted, cast to bf16
            nc.vector.scalar_tensor_tensor(qn[:, :], pj[:, :], w_ap, rec[:, :],
                                           op0=mult, op1=mult)
            # rope: out = qn*cos2 + swap_halves(qn)*[-sin; sin]
            cs = cosT[:, sg * 512:(sg + 1) * 512]
            sn = sinT[:, sg * 512:(sg + 1) * 512]
            sw = swpp.tile([128, 512], F32, tag="sw")
            nc.tensor.matmul(sw[:, :], swp_t[:, :], qn[:, :], start=True, stop=True)
            t1 = tmps.tile([128, 512], BF16, tag="t1")
            t2 = tmps.tile([128, 512], BF16, tag="t2")
            nc.vector.tensor_mul(t1[:, :], qn[:, :], cs)
            nc.vector.tensor_mul(t2[:, :], sw[:, :], sn)
            nc.vector.tensor_add(dst[:, dst_cols], t1[:, :], t2[:, :])

        with tc.tile_pool(name="xp", bufs=44) as xp, \
             tc.tile_pool(name="wp", bufs=16) as wp, \
             tc.tile_pool(name="wkvp", bufs=84) as wkvp, \
             tc.tile_pool(name="sqp", bufs=2) as sqp, \
             tc.tile_pool(name="stats", bufs=4) as stats, \
             tc.tile_pool(name="tmps", bufs=4) as tmps, \
             tc.tile_pool(name="pj", bufs=2, space="PSUM") as pjp, \
             tc.tile_pool(name="ssp", bufs=1, space="PSUM") as ssp, \
             tc.tile_pool(name="bcp", bufs=1, space="PSUM") as bcp, \
             tc.tile_pool(name="swpp", bufs=2, space="PSUM") as swpp, \
             tc.tile_pool(name="pv", bufs=2, space="PSUM") as pvp:
            for sg in range(NG):
                xts = []
                for dc in range(DC):
                    xt = xp.tile([128, 512], BF16, tag="xt")
                    nc.sync.dma_start(out=xt[:, :], in_=xt_e[dc, sg])
                    xts.append(xt)
                # K projection (one 128-block) then V then Q heads
                wks = []
                wvs = []
                for dc in range(DC):
                    wk_t = wkvp.tile([128, 128], BF16, tag="wkv")
                    nc.sync.dma_start(out=wk_t[:, :], in_=wk_e[dc])
                    wks.append(wk_t)
                    wv_t = wkvp.tile([128, 128], BF16, tag="wkv")
                    nc.sync.dma_start(out=wv_t[:, :], in_=wv_e[dc])
                    wvs.append(wv_t)
                cols = slice(sg * 512, (sg + 1) * 512)
                pk = pjp.tile([128, 512], F32, tag="pj")
                for dc in range(DC):
                    nc.tensor.matmul(pk[:, :], wks[dc][:, :], xts[dc][:, :],
                                     start=(dc == 0), stop=(dc == DC - 1))
                norm_rope(pk, kw_t[:, :], KR, cols, sg, sqp, ssp, bcp, swpp,
                          stats, tmps)
                # V in [s, d]: psum [128 s, 128 d] per s-block of this group
                for sb4 in range(4):
                    sb = sg * 4 + sb4
                    pv = pvp.tile([128, 128], F32, tag="pv")
                    for dc in range(DC):
                        nc.tensor.matmul(
                            pv[:, :], xts[dc][:, sb4 * 128:(sb4 + 1) * 128],
                            wvs[dc][:, :],
                            start=(dc == 0), stop=(dc == DC - 1))
                    nc.vector.tensor_copy(Vsd[:, sb * 128:(sb + 1) * 128], pv[:, :])
                # Q heads
                for qb in range(HQ):
                    wqs = []
                    for dc in range(DC):
                        wq_t = wp.tile([128, 128], BF16, tag="wq")
                        nc.sync.dma_start(out=wq_t[:, :], in_=wq_e[dc, qb])
                        wqs.append(wq_t)
                    pq = pjp.tile([128, 512], F32, tag="pj")
                    for dc in range(DC):
                        nc.tensor.matmul(pq[:, :], wqs[dc][:, :], xts[dc][:, :],
                                         start=(dc == 0), stop=(dc == DC - 1))
                    qcols = slice(qb * S + sg * 512, qb * S + (sg + 1) * 512)
                    norm_rope(pq, qw_t[:, :], QR, qcols, sg, sqp, ssp, bcp, swpp,
                              stats, tmps)

        # ---- stage 3: attention per head ---------------------------------
        with tc.tile_pool(name="ptp", bufs=4) as ptp, \
             tc.tile_pool(name="mgp", bufs=8) as mgp, \
             tc.tile_pool(name="aeps", bufs=6) as aeps, \
             tc.tile_pool(name="st", bufs=2, space="PSUM") as stp, \
             tc.tile_pool(name="ot", bufs=2, space="PSUM") as otp, \
             tc.tile_pool(name="rsp", bufs=2, space="PSUM") as rsp, \
             tc.tile_pool(name="abcp", bufs=2, space="PSUM") as abcp:
            for h in range(HQ):
                for qg in range(NG):
                    qsl = slice(h * S + qg * 512, h * S + (qg + 1) * 512)
                    nkb = (qg + 1) * 4 if causal else SB
                    ot = otp.tile([128, 512], F32, tag="ot")
                    rs = rsp.tile([1, 512], F32, tag="rs")
                    for kb in range(nkb):
                        # causal: only q >= kb*128 can attend to this k block
                        c0 = max(0, kb * 128 - qg * 512) if causal else 0
                        w = 512 - c0
                        q0 = h * S + qg * 512 + c0
                        st = stp.tile([128, 512], F32, tag="st")
                        nc.tensor.matmul(st[:, c0:], 
                                         KR[:, kb * 128:(kb + 1) * 128],
                                         QR[:, q0:q0 + w], start=True, stop=True)
                        if causal:
                            if kb >= qg * 4:  # diagonal block of this q group
                                nc.vector.tensor_add(
                                    st[:, c0:c0 + 128], st[:, c0:c0 + 128],
                                    maskT[:, kb * 128:(kb + 1) * 128])
                        else:
                            mt = mgp.tile([128, 512], F32, tag="mg")
                            nc.sync.dma_start(out=mt[:, :], in_=mask_e[kb, qg])
                            nc.vector.tensor_add(st[:, :], st[:, :], mt[:, :])
                        pt = ptp.tile([128, 512], BF16, tag="pt")
                        nc.scalar.activation(pt[:, c0:], st[:, c0:], Exp)
                        nc.tensor.matmul(rs[:, c0:], ones_c[:, :], pt[:, c0:],
                                         start=(kb == 0), stop=(kb == nkb - 1),
                                         skip_group_check=True)
                        nc.tensor.matmul(ot[:, c0:],
                                         Vsd[:, kb * 128:(kb + 1) * 128],
                                         pt[:, c0:],
                                         start=(kb == 0), stop=(kb == nkb - 1),
                                         skip_group_check=True)
                    rsb = aeps.tile([1, 512], BF16, tag="rsb")
                    nc.vector.tensor_copy(rsb[:, :], rs[:, :])
                    bc = abcp.tile([128, 512], F32, tag="abc")
                    nc.tensor.matmul(bc[:, :], ones_r[:, :], rsb[:, :],
                                     start=True, stop=True)
                    rec = aeps.tile([128, 512], F32, tag="arec")
                    nc.vector.reciprocal(rec[:, :], bc[:, :])
                    nc.vector.tensor_mul(OT[:, qsl], ot[:, :], rec[:, :])

        # ---- stage 4: wo projection + ReduceScatter ----------------------
        with tc.tile_pool(name="wop", bufs=80) as wop, \
             tc.tile_pool(name="oep", bufs=3) as oep, \
             tc.tile_pool(name="po", bufs=2, space="PSUM") as pop, \
             tc.tile_pool(name="dram", bufs=1, space="DRAM") as dram:
            wos = {}
            for c in range(HQ):
                for dg in range(DG):
                    wo_t = wop.tile([128, 512], BF16, tag="wo")
                    nc.sync.dma_start(out=wo_t[:, :], in_=wo_e[c, dg])
                    wos[(c, dg)] = wo_t
            partial = dram.tile([S, D], F32)
            rs_out = dram.tile([S // N_CORES, D], F32)
            for sb in range(SB):
                for dg in range(DG):
                    po = pop.tile([128, 512], F32, tag="po")
                    for c in range(HQ):
                        nc.tensor.matmul(
                            po[:, :],
                            OT[:, c * S + sb * 128:c * S + (sb + 1) * 128],
                            wos[(c, dg)][:, :],
                            start=(c == 0), stop=(c == HQ - 1))
                    ob = oep.tile([128, 512], F32, tag="ob")
                    nc.vector.tensor_copy(ob[:, :], po[:, :])
                    nc.sync.dma_start(
                        out=partial[sb * 128:(sb + 1) * 128,
                                    dg * 512:(dg + 1) * 512],
                        in_=ob[:, :])
            nc.gpsimd.collective_compute(
                "ReduceScatter",
                mybir.AluOpType.add,
                replica_groups=[list(range(N_CORES))],
                ins=[partial.opt()],
                outs=[rs_out.opt()],
            )
            nc.sync.dma_start(out=out_e[:, :], in_=rs_out[:, :])

    nc.compile()
    return nc


def _host_prep(x, wq, wk, wv, wo, q_norm_w, k_norm_w, freqs_cos, freqs_sin,
               mask, causal):
    xs = x[0]                                    # [S, D] f32
    xt = np.ascontiguousarray(xs.T)              # [D, S]
    xt_t = xt.reshape(DC, 128, NG, 512).transpose(0, 2, 1, 3)
    xt_t = np.ascontiguousarray(xt_t).astype(NPBF16)

    p = np.concatenate([np.arange(0, HD, 2), np.arange(1, HD, 2)])
    c64 = np.ascontiguousarray(freqs_cos.T)                   # [64, S]
    s64 = np.ascontiguousarray(freqs_sin.T)
    cosT = np.concatenate([c64, c64], axis=0).astype(NPBF16)  # [128, S]
    sinT = np.concatenate([-s64, s64], axis=0).astype(NPBF16)
    swp = np.zeros((HD, HD), dtype=np.float32)
    swp[np.arange(HD), np.arange(HD) ^ 64] = 1.0
    swp = swp.astype(NPBF16)

    if causal:
        mask_t = np.stack([
            np.ascontiguousarray(mask[b * 128:(b + 1) * 128,
                                      b * 128:(b + 1) * 128].T)
            for b in range(SB)
        ]).astype(np.float32)
    else:
        mt = np.ascontiguousarray(mask.T)        # [k, q]
        mask_t = np.ascontiguousarray(
            mt.reshape(SB, 128, NG, 512).transpose(0, 2, 1, 3)).astype(np.float32)

    in_maps = []
    for c in range(N_CORES):
        wq_s = wq[c * HQ * HD:(c + 1) * HQ * HD].reshape(HQ, HD, D)[:, p]
        wqT = np.ascontiguousarray(wq_s.reshape(HQ * HD, D).T)   # [D, 1024]
        wq_t = np.ascontiguousarray(
            wqT.reshape(DC, 128, HQ, 128).transpose(0, 2, 1, 3)).astype(NPBF16)
        wkT = np.ascontiguousarray(wk[c * HD:(c + 1) * HD][p].T)  # [D, 128]
        wk_t = np.ascontiguousarray(wkT.reshape(DC, 128, 128)).astype(NPBF16)
        wvT = np.ascontiguousarray(wv[c * HD:(c + 1) * HD].T)
        wv_t = np.ascontiguousarray(wvT.reshape(DC, 128, 128)).astype(NPBF16)
        woT = np.ascontiguousarray(wo[:, c * HQ * HD:(c + 1) * HQ * HD].T)
        wo_t = np.ascontiguousarray(
            woT.reshape(HQ, 128, DG, 512).transpose(0, 2, 1, 3)).astype(NPBF16)
        qw_v = (q_norm_w[p] / np.sqrt(HD)).astype(np.float32).reshape(HD, 1)
        kw_v = k_norm_w[p].astype(np.float32).reshape(HD, 1)
        in_maps.append({
            "xt": xt_t, "wq": wq_t, "wk": wk_t, "wv": wv_t, "wo": wo_t,
            "cos": cosT, "sin": sinT, "swp": swp, "qw": qw_v, "kw": kw_v,
            "mask": mask_t,
        })
    return in_maps


def _numpy_fallback(x, wq, wk, wv, wo, q_norm_w, k_norm_w, cache_k, cache_v,
                    freqs_cos, freqs_sin, mask, start_pos):
    bsz, seqlen, _ = x.shape
    xq = (x @ wq.T).reshape(bsz, seqlen, H, HD)
    xk = (x @ wk.T).reshape(bsz, seqlen, KVH, HD)
    xv = (x @ wv.T).reshape(bsz, seqlen, KVH, HD)

    def rms(v, w):
        n = v * (1.0 / np.sqrt((v * v).mean(-1, keepdims=True) + EPS))
        return n * w

    def rope(v):
        vr = v.reshape(*v.shape[:-1], HD // 2, 2)
        ve, vo = vr[..., 0], vr[..., 1]
        c = freqs_cos[None, :, None, :]
        s = freqs_sin[None, :, None, :]
        oe = ve * c - vo * s
        oo = ve * s + vo * c
        return np.stack([oe, oo], axis=-1).reshape(v.shape)

    xq = rope(rms(xq, q_norm_w))
    xk = rope(rms(xk, k_norm_w))
    ck = np.array(cache_k)
    cv = np.array(cache_v)
    ck[:bsz, start_pos:start_pos + seqlen] = xk
    cv[:bsz, start_pos:start_pos + seqlen] = xv
    kv_len = start_pos + seqlen
    keys = np.repeat(ck[:bsz, :kv_len], H // KVH, axis=2)
    values = np.repeat(cv[:bsz, :kv_len], H // KVH, axis=2)
    sc = np.einsum('bqhd,bkhd->bhqk', xq, keys) / np.sqrt(HD)
    if mask is not None:
        sc = sc + mask[None, None, :, :]
    sc = sc - sc.max(-1, keepdims=True)
    e = np.exp(sc)
    probs = e / e.sum(-1, keepdims=True)
    out = np.einsum('bhqk,bkhd->bqhd', probs, values)
    return (out.reshape(bsz, seqlen, H * HD) @ wo.T).astype(np.float32)


def _run(trace=False, **inputs):
    x = np.asarray(inputs["x"], dtype=np.float32)
    wq = np.asarray(inputs["wq"], dtype=np.float32)
    wk = np.asarray(inputs["wk"], dtype=np.float32)
    wv = np.asarray(inputs["wv"], dtype=np.float32)
    wo = np.asarray(inputs["wo"], dtype=np.float32)
    q_norm_w = np.asarray(inputs["q_norm_w"], dtype=np.float32)
    k_norm_w = np.asarray(inputs["k_norm_w"], dtype=np.float32)
    freqs_cos = np.asarray(inputs["freqs_cos"], dtype=np.float32)
    freqs_sin = np.asarray(inputs["freqs_sin"], dtype=np.float32)
    mask = np.asarray(inputs["mask"], dtype=np.float32)
    start_pos = int(inputs.get("start_pos", 0))

    if start_pos != 0 or x.shape != (1, S, D):
        return _numpy_fallback(
            x, wq, wk, wv, wo, q_norm_w, k_norm_w,
            np.asarray(inputs["cache_k"]), np.asarray(inputs["cache_v"]),
            freqs_cos, freqs_sin, mask, start_pos), None

    causal = bool(
        (mask == np.triu(np.full((S, S), -1e9, dtype=np.float32), k=1)).all())

    key = ("nc", causal)
    if key not in _cache:
        _cache[key] = _build(causal)
    nc = _cache[key]
    in_maps = _host_prep(x, wq, wk, wv, wo, q_norm_w, k_norm_w,
                         freqs_cos, freqs_sin, mask, causal)
    res = run_bass_kernel_spmd(nc, in_maps, core_ids=list(range(N_CORES)),
                               trace=trace)
    out = np.concatenate([res.results[c]["out"] for c in range(N_CORES)],
                         axis=0)
    return out.reshape(1, S, D).astype(np.float32), res


def kernel(**inputs) -> np.ndarray:
    out, _ = _run(trace=False, **inputs)
    return out
